# revision 1
# baseline (speedup 1.0000x reference)
"""Linformer attention TRN2 kernel (8 NeuronCores), v2.

Sharding: core c handles batch b = c//2 and head-half hh = c%2
(8 of 16 heads = 512 of 1024 feature columns of Wq/Wk/Wv, and the
matching 512 rows of Wo). Each core computes a partial output
y_part = attn_out_half @ Wo[hh*512:(hh+1)*512, :]; the host sums the
two partials per batch and adds bo.

Per-core dataflow (L=4096, D=1024, 8 heads x hd=64, K=256):
  phase 1 (stream L in slices of 512): k/v projections (bias folded via
  broadcast-row adds on DVE), then
    kET += k_headpair^T @ E   (PSUM-resident accumulators)
    vF  += F_chunk^T @ v      (PSUM-resident accumulators)
  DMAs are emitted in first-use order (xT/wk/wv chunks interleaved) so
  the first matmul starts ~6us in.
  phase 2 (per L-tile of 512), software-pipelined:
    qT(lt) = Wq_s^T @ xT(lt)  (xT slice re-DMAed; computed 1 tile
             ahead; drain on ACT with fused bias+scale)
    scores(h) = qT_h^T @ kET_h      -> f32 PSUM [128l, 4j, 256k]
    softmax: per-j rowmax (DVE) -> exp with accum_out=Z (ACT, per-j
             bias) -> recip + normalize (DVE)
    sT = PE-transpose(s2) -> bf16 PSUM -> DVE drain
    outT_h = vF_h^T @ sT  (PSUM) -> drain alternating DVE/ACT
    y(lt) = outT^T @ Wo_s (emitted during NEXT tile's scores) ->
            drains split DVE/ACT -> DMA
  Emission spreads the 8 heads' scores across the whole tile body at
  the rate ACT can exp-drain them (2 score PSUM buffers), with
  y(lt-1), q(lt+1) and transposes filling the PE gaps.
"""

import sys

sys.path.insert(0, "/opt/trn_rl_repo")

import numpy as np
import ml_dtypes

import concourse.bass as bass
import concourse.mybir as mybir
import concourse.tile as tile
from concourse import bacc
from concourse import bass_utils

B, L, D, H, HD, K = 4, 4096, 1024, 16, 64, 256
DH = 512                      # per-core feature slice (8 heads x 64)
NHL = 8                       # heads per core
SCALE = HD ** -0.5
P = 128
LS1 = 512                     # phase-1 L slice
NS1 = L // LS1                # 8 phase-1 iterations
LT2 = 512                     # phase-2 L tile
NT2 = L // LT2                # 8 phase-2 iterations
F32 = mybir.dt.float32
F32R = mybir.dt.float32r
BF16 = mybir.dt.bfloat16
FP16 = mybir.dt.float16

MMDT = F32R                   # dtype of every matmul-feeding tensor

_CACHE = {}


def build_program():
    nc = bacc.Bacc("TRN2", target_bir_lowering=False, debug=False)

    xt = nc.dram_tensor("xt", [D, L], MMDT, kind="ExternalInput").ap()
    wq = nc.dram_tensor("wq", [D, DH], MMDT, kind="ExternalInput").ap()
    wk = nc.dram_tensor("wk", [D, DH], MMDT, kind="ExternalInput").ap()
    wv = nc.dram_tensor("wv", [D, DH], MMDT, kind="ExternalInput").ap()
    wo = nc.dram_tensor("wo", [DH, D], MMDT, kind="ExternalInput").ap()
    bqs = nc.dram_tensor("bqs", [P, DH // P], F32, kind="ExternalInput").ap()
    bkr = nc.dram_tensor("bkr", [P, DH], F32, kind="ExternalInput").ap()
    bvr = nc.dram_tensor("bvr", [P, DH], F32, kind="ExternalInput").ap()
    Edr = nc.dram_tensor("E", [L, K], MMDT, kind="ExternalInput").ap()
    Fdr = nc.dram_tensor("F", [L, K], MMDT, kind="ExternalInput").ap()
    idbr = nc.dram_tensor("identb", [P, P], BF16, kind="ExternalInput").ap()
    ydr = nc.dram_tensor("y", [L, D], F32, kind="ExternalOutput").ap()

    with tile.TileContext(nc) as tc:
        with (
            tc.tile_pool(name="const", bufs=1) as constp,
            tc.tile_pool(name="persist", bufs=1) as persist,
            tc.tile_pool(name="w2", bufs=1) as w2p,
        ):
            identb_t = constp.tile([P, P], BF16, name="identb_t")
            nc.sync.dma_start(identb_t[:], idbr)
            identb = identb_t[:]
            bqs_sb = constp.tile([P, DH // P], F32)
            nc.sync.dma_start(bqs_sb[:], bqs)
            bkr_sb = constp.tile([P, DH], F32)
            nc.sync.dma_start(bkr_sb[:], bkr)
            bvr_sb = constp.tile([P, DH], F32)
            nc.sync.dma_start(bvr_sb[:], bvr)

            # persistent SBUF tensors
            kET_sb = persist.tile([P, 4, K], MMDT)      # [pair-row, pair, k]
            vF_sb = persist.tile([P, 2, DH], BF16)      # [k-in-chunk, kc, h*64+d]

            # phase-2 weights (DMAs emitted later, during phase-1 ls 0/1)
            wq_sb = w2p.tile([P, D // P, DH], MMDT, tag="wq")
            wo_sb = w2p.tile([P, DH // P, D], MMDT, tag="wo")

            # ---------------- phase 1: k/v -> kET, vF ----------------
            with (
                tc.tile_pool(name="w1", bufs=1) as w1,
                tc.tile_pool(name="xtp", bufs=2) as xtp,
                tc.tile_pool(name="kvp", bufs=2) as kvp,
                tc.tile_pool(name="efp", bufs=2) as efp,
                tc.tile_pool(name="ps_acc", bufs=1, space="PSUM") as ps_acc,
                tc.tile_pool(name="ps_mm1", bufs=4, space="PSUM") as ps_mm1,
            ):
                # first slice's inputs go out on the wire before the big
                # weight loads so the PE can start within ~8us
                # interleave first-slice and weight-chunk DMAs in the
                # order the first MM groups consume them
                xT0 = xtp.tile([P, D // P, LS1], MMDT)
                wk_sb = w1.tile([P, D // P, DH], MMDT, tag="wk")
                wv_sb = w1.tile([P, D // P, DH], MMDT, tag="wv")
                for dc in range(D // P):
                    nc.sync.dma_start(
                        xT0[:, dc, :], xt[dc * P : (dc + 1) * P, 0:LS1]
                    )
                    nc.sync.dma_start(wk_sb[:, dc, :], wk[dc * P : (dc + 1) * P, :])
                    nc.sync.dma_start(wv_sb[:, dc, :], wv[dc * P : (dc + 1) * P, :])
                e0 = efp.tile([P, LS1 // P, K], MMDT, tag="E")
                nc.sync.dma_start(
                    e0[:], Edr[0:LS1, :].rearrange("(a p) k -> p a k", p=P)
                )
                f0 = efp.tile([P, LS1 // P, K], MMDT, tag="F")
                nc.sync.dma_start(
                    f0[:], Fdr[0:LS1, :].rearrange("(a p) k -> p a k", p=P)
                )

                # persistent PSUM accumulators
                kET_ps = [
                    ps_acc.tile([P, 2 * K], F32, tag=f"kET{i}", name=f"kET_ps{i}")
                    for i in range(2)
                ]
                vF_ps = [
                    ps_acc.tile([P, DH], F32, tag=f"vF{i}", name=f"vF_ps{i}")
                    for i in range(2)
                ]

                for ls in range(NS1):
                    l0 = ls * LS1
                    # x^T slice [d, l] comes pre-transposed from the host
                    if ls == 0:
                        xT = xT0
                    else:
                        xT = xtp.tile([P, D // P, LS1], MMDT)
                        for dc in range(D // P):
                            nc.sync.dma_start(
                                xT[:, dc, :],
                                xt[dc * P : (dc + 1) * P, l0 : l0 + LS1],
                            )
                    # k, v natural layout slices
                    k_sl = kvp.tile([P, LS1 // P, DH], MMDT, tag="k")
                    v_sl = kvp.tile([P, LS1 // P, DH], MMDT, tag="v")
                    # E / F slices
                    if ls == 0:
                        e_sl, f_sl = e0, f0
                    else:
                        e_sl = efp.tile([P, LS1 // P, K], MMDT, tag="E")
                        nc.sync.dma_start(
                            e_sl[:],
                            Edr[l0 : l0 + LS1, :].rearrange("(a p) k -> p a k", p=P),
                        )
                        f_sl = efp.tile([P, LS1 // P, K], MMDT, tag="F")
                        nc.sync.dma_start(
                            f_sl[:],
                            Fdr[l0 : l0 + LS1, :].rearrange("(a p) k -> p a k", p=P),
                        )
                        # phase-2 weights ride behind the streaming inputs
                        if ls == 2:
                            nc.sync.dma_start(
                                wq_sb[:], wq.rearrange("(c p) n -> p c n", p=P)
                            )
                        elif ls == 3:
                            nc.sync.dma_start(
                                wo_sb[:], wo.rearrange("(c p) n -> p c n", p=P)
                            )
                    first = ls == 0
                    last = ls == NS1 - 1

                    def acc_a(a):
                        # kET accumulation: 4 head-pairs, 2 pairs share a
                        # PSUM bank -> only one start/stop per bank.
                        for pr in range(4):
                            nc.tensor.matmul(
                                kET_ps[pr // 2][:, (pr % 2) * K : (pr % 2 + 1) * K],
                                k_sl[:, a, pr * P : (pr + 1) * P],
                                e_sl[:, a, :],
                                start=(first and a == 0 and pr % 2 == 0),
                                stop=(last and a == LS1 // P - 1 and pr % 2 == 1),
                            )
                        # vF accumulation: 2 K-chunks
                        for kc in range(2):
                            nc.tensor.matmul(
                                vF_ps[kc][:],
                                f_sl[:, a, kc * P : (kc + 1) * P],
                                v_sl[:, a, :],
                                start=(first and a == 0),
                                stop=(last and a == LS1 // P - 1),
                            )

                    for a in range(LS1 // P):
                        pk = ps_mm1.tile([P, DH], F32, tag="mm1")
                        for dc in range(D // P):
                            nc.tensor.matmul(
                                pk[:],
                                xT[:, dc, a * P : (a + 1) * P],
                                wk_sb[:, dc, :],
                                start=(dc == 0),
                                stop=(dc == D // P - 1),
                            )
                        nc.vector.tensor_add(k_sl[:, a, :], pk[:], bkr_sb[:])
                        pv = ps_mm1.tile([P, DH], F32, tag="mm1")
                        for dc in range(D // P):
                            nc.tensor.matmul(
                                pv[:],
                                xT[:, dc, a * P : (a + 1) * P],
                                wv_sb[:, dc, :],
                                start=(dc == 0),
                                stop=(dc == D // P - 1),
                            )
                        nc.vector.tensor_add(v_sl[:, a, :], pv[:], bvr_sb[:])
                        # accumulate the previous a-block (its drains done)
                        if a >= 1:
                            acc_a(a - 1)
                    acc_a(LS1 // P - 1)
                # copy accumulators to SBUF
                for i in range(2):
                    nc.scalar.copy(
                        kET_sb[:, 2 * i : 2 * i + 2, :],
                        kET_ps[i][:].rearrange("p (c n) -> p c n", n=K),
                    )
                    nc.scalar.copy(vF_sb[:, i, :], vF_ps[i][:])

            # ---------------- phase 2 ----------------
            with (
                tc.tile_pool(name="xt2", bufs=2) as xt2p,
                tc.tile_pool(name="qt", bufs=2) as qtp,
                tc.tile_pool(name="s2p", bufs=8) as s2p,
                tc.tile_pool(name="sTp", bufs=8) as sTp,
                tc.tile_pool(name="otp", bufs=3) as otp,
                tc.tile_pool(name="yp", bufs=4) as yp,
                tc.tile_pool(name="stat", bufs=6) as stat,
                tc.tile_pool(name="ps_q", bufs=1, space="PSUM") as ps_q,
                tc.tile_pool(name="ps_sc", bufs=2, space="PSUM") as ps_sc,
                tc.tile_pool(name="ps_pt", bufs=1, space="PSUM") as ps_pt,
                tc.tile_pool(name="ps_out", bufs=2, space="PSUM") as ps_out,
            ):

                def emit_q_rc(xt_tile, qT_tile, rc):
                    pq = ps_q.tile([P, LT2], F32, tag="pq", name="pq")
                    for dc in range(D // P):
                        nc.tensor.matmul(
                            pq[:],
                            wq_sb[:, dc, rc * P : (rc + 1) * P],
                            xt_tile[:, dc, :],
                            start=(dc == 0),
                            stop=(dc == D // P - 1),
                        )
                    # qT = SCALE*(x@Wq) + SCALE*bq  (bqs pre-scaled on host)
                    nc.scalar.activation(
                        qT_tile[:, rc, :],
                        pq[:],
                        mybir.ActivationFunctionType.Identity,
                        bias=bqs_sb[:, rc : rc + 1],
                        scale=SCALE,
                    )

                def emit_scores(h, qT_t):
                    po = (h % 2) * HD
                    pair = h // 2
                    ps2 = ps_sc.tile([P, 4, K], F32, tag="sc", name="ps2")
                    for j in range(4):
                        nc.tensor.matmul(
                            ps2[:, j, :],
                            qT_t[po : po + HD, pair,
                                 j * P : (j + 1) * P],
                            kET_sb[po : po + HD, pair, :],
                            start=True,
                            stop=True,
                        )
                    negm = stat.tile([P, 4], F32, tag="negm")
                    nc.vector.reduce_max(
                        negm[:], ps2[:], axis=mybir.AxisListType.X, negate=True
                    )
                    s2 = s2p.tile([P, 4, K], BF16, tag="s2")
                    z = stat.tile([P, 4], F32, tag="z")
                    for j in range(4):
                        nc.scalar.activation(
                            s2[:, j, :],
                            ps2[:, j, :],
                            mybir.ActivationFunctionType.Exp,
                            bias=negm[:, j : j + 1],
                            scale=1.0,
                            accum_out=z[:, j : j + 1],
                        )
                    return s2, z

                def emit_zrm(s2, z):
                    rz = stat.tile([P, 4], F32, tag="rz")
                    nc.vector.reciprocal(rz[:], z[:])
                    for j in range(4):
                        nc.vector.tensor_scalar_mul(
                            s2[:, j, :], s2[:, j, :], rz[:, j : j + 1]
                        )

                def emit_transp(h, s2):
                    pt = ps_pt.tile([P, 2, LT2], BF16, tag="pt", name="pt")
                    for j in range(4):
                        for kc in range(2):
                            nc.tensor.transpose(
                                pt[:, kc, j * P : (j + 1) * P],
                                s2[:, j, kc * P : (kc + 1) * P],
                                identb,
                            )
                    sT = sTp.tile([P, 2, LT2], BF16, tag="sT")
                    nc.vector.tensor_copy(sT[:], pt[:])
                    return sT

                def emit_pv(h, sT, outT_t):
                    po = (h % 2) * HD
                    pair = h // 2
                    pp = ps_out.tile([HD, LT2], F32, tag="out", name="pp")
                    for kc in range(2):
                        nc.tensor.matmul(
                            pp[:],
                            vF_sb[:, kc, h * HD : (h + 1) * HD],
                            sT[:, kc, :],
                            start=(kc == 0),
                            stop=(kc == 1),
                        )
                    if h % 2 == 0:
                        nc.vector.tensor_copy(outT_t[po : po + HD, pair, :], pp[:])
                    else:
                        nc.scalar.copy(outT_t[po : po + HD, pair, :], pp[:])

                def emit_y(j, outT_t, l0):
                    y_sb = yp.tile([P, D], F32, tag="y")
                    for nh in range(2):
                        py = ps_out.tile([P, DH], F32, tag="out", name="py")
                        for c in range(4):
                            nc.tensor.matmul(
                                py[:],
                                outT_t[:, c, j * P : (j + 1) * P],
                                wo_sb[:, c, nh * DH : (nh + 1) * DH],
                                start=(c == 0),
                                stop=(c == 3),
                            )
                        if nh == 0:
                            nc.vector.tensor_copy(
                                y_sb[:, nh * DH : (nh + 1) * DH], py[:]
                            )
                        else:
                            nc.scalar.copy(y_sb[:, nh * DH : (nh + 1) * DH], py[:])
                    nc.sync.dma_start(ydr[l0 + j * P : l0 + (j + 1) * P, :], y_sb[:])

                # prologue: q(0)
                xt2_next = xt2p.tile([P, D // P, LT2], MMDT, tag="xt2")
                nc.sync.dma_start(
                    xt2_next[:], xt[:, 0:LT2].rearrange("(c p) l -> p c l", p=P)
                )
                qT_next = qtp.tile([P, 4, LT2], MMDT, tag="qt")
                for rc in range(4):
                    emit_q_rc(xt2_next, qT_next, rc)
                qT_cur = qT_next

                outT_prev = None
                l0_prev = 0
                for lt in range(NT2):
                    l0 = lt * LT2
                    if lt + 1 < NT2:
                        xt2_next = xt2p.tile([P, D // P, LT2], MMDT, tag="xt2")
                        nc.sync.dma_start(
                            xt2_next[:],
                            xt[:, l0 + LT2 : l0 + 2 * LT2].rearrange(
                                "(c p) l -> p c l", p=P
                            ),
                        )
                    have_q = lt + 1 < NT2
                    have_y = outT_prev is not None
                    if have_q:
                        qT_next = qtp.tile([P, 4, LT2], MMDT, tag="qt")

                    # spread schedule: scores trickle out at the rate ACT
                    # can exp-drain them (2 PSUM score buffers); transposes,
                    # q(lt+1) and y(lt-1) fill the PE gaps.
                    s2s = []
                    sTs = []
                    s2s.append(emit_scores(0, qT_cur))
                    s2s.append(emit_scores(1, qT_cur))
                    if have_y:
                        emit_y(0, outT_prev, l0_prev)
                        emit_y(1, outT_prev, l0_prev)
                    s2s.append(emit_scores(2, qT_cur))
                    emit_zrm(*s2s[0])
                    sTs.append(emit_transp(0, s2s[0][0]))
                    s2s.append(emit_scores(3, qT_cur))
                    emit_zrm(*s2s[1])
                    sTs.append(emit_transp(1, s2s[1][0]))
                    outT_t = otp.tile([P, 4, LT2], MMDT, tag="outT")
                    s2s.append(emit_scores(4, qT_cur))
                    emit_zrm(*s2s[2])
                    sTs.append(emit_transp(2, s2s[2][0]))
                    if have_q:
                        emit_q_rc(xt2_next, qT_next, 0)
                    emit_pv(0, sTs[0], outT_t)
                    s2s.append(emit_scores(5, qT_cur))
                    emit_zrm(*s2s[3])
                    sTs.append(emit_transp(3, s2s[3][0]))
                    if have_q:
                        emit_q_rc(xt2_next, qT_next, 1)
                    emit_pv(1, sTs[1], outT_t)
                    s2s.append(emit_scores(6, qT_cur))
                    emit_zrm(*s2s[4])
                    sTs.append(emit_transp(4, s2s[4][0]))
                    if have_q:
                        emit_q_rc(xt2_next, qT_next, 2)
                    emit_pv(2, sTs[2], outT_t)
                    s2s.append(emit_scores(7, qT_cur))
                    emit_zrm(*s2s[5])
                    sTs.append(emit_transp(5, s2s[5][0]))
                    if have_q:
                        emit_q_rc(xt2_next, qT_next, 3)
                    emit_pv(3, sTs[3], outT_t)
                    emit_zrm(*s2s[6])
                    sTs.append(emit_transp(6, s2s[6][0]))
                    emit_pv(4, sTs[4], outT_t)
                    emit_zrm(*s2s[7])
                    sTs.append(emit_transp(7, s2s[7][0]))
                    emit_pv(5, sTs[5], outT_t)
                    emit_pv(6, sTs[6], outT_t)
                    emit_pv(7, sTs[7], outT_t)
                    # y(lt-1) j2-3 (executes while PV of this tile runs)
                    if have_y:
                        emit_y(2, outT_prev, l0_prev)
                        emit_y(3, outT_prev, l0_prev)
                    outT_prev = outT_t
                    l0_prev = l0
                    if have_q:
                        qT_cur = qT_next
                # epilogue: y(7)
                for j in range(4):
                    emit_y(j, outT_prev, l0_prev)
    nc.compile()
    return nc


def _get_program():
    if "nc" not in _CACHE:
        _CACHE["nc"] = build_program()
    return _CACHE["nc"]


def _shard_inputs(inputs):
    x = np.asarray(inputs["x"], np.float32)
    Wq = np.asarray(inputs["Wq"], np.float32)
    bq = np.asarray(inputs["bq"], np.float32)
    Wk = np.asarray(inputs["Wk"], np.float32)
    bk = np.asarray(inputs["bk"], np.float32)
    Wv = np.asarray(inputs["Wv"], np.float32)
    bv = np.asarray(inputs["bv"], np.float32)
    E = np.ascontiguousarray(np.asarray(inputs["E"], np.float32))
    F = np.ascontiguousarray(np.asarray(inputs["F"], np.float32))
    Wo = np.asarray(inputs["Wo"], np.float32)
    in_maps = []
    for c in range(8):
        b, hh = c // 2, c % 2
        sl = slice(hh * DH, (hh + 1) * DH)
        in_maps.append(
            {
                "xt": np.ascontiguousarray(x[b].T),
                "wq": np.ascontiguousarray(Wq[:, sl]),
                "wk": np.ascontiguousarray(Wk[:, sl]),
                "wv": np.ascontiguousarray(Wv[:, sl]),
                "wo": np.ascontiguousarray(Wo[sl, :]),
                "bqs": np.ascontiguousarray(
                    (bq[sl] * SCALE).reshape(4, P).T.astype(np.float32)
                ),
                "bkr": np.ascontiguousarray(np.broadcast_to(bk[sl], (P, DH))),
                "bvr": np.ascontiguousarray(np.broadcast_to(bv[sl], (P, DH))),
                "E": E,
                "F": F,
                "identb": np.eye(P, dtype=ml_dtypes.bfloat16),
            }
        )
    return in_maps


def _ensure_profile_hook():
    """The container's `antenv` stub lacks `axon_hooks`; synthesize it so
    run_bass_kernel_spmd(trace=True) can reach the NTFF capture ABI in
    libaxon_pjrt.so (see trn_agent_boot.trn_boot)."""
    import types
    import antenv

    if hasattr(antenv, "axon_hooks"):
        return
    mod = types.ModuleType("antenv.axon_hooks")
    _state = {"hook": None}
    mod.set_axon_ntff_profile_hook = lambda h: _state.__setitem__("hook", h)
    mod.get_axon_ntff_profile_hook = lambda: _state["hook"]
    sys.modules["antenv.axon_hooks"] = mod
    antenv.axon_hooks = mod
    try:
        from trn_agent_boot.trn_boot import _ntff_profile_via_ctypes

        mod.set_axon_ntff_profile_hook(
            _ntff_profile_via_ctypes("/opt/axon/libaxon_pjrt.so")
        )
    except Exception as e:
        print(f"profile hook setup failed: {e}", file=sys.stderr)


def run(inputs, trace=False, **kw):
    if trace:
        _ensure_profile_hook()
    nc = _get_program()
    in_maps = _shard_inputs(inputs)
    res = bass_utils.run_bass_kernel_spmd(
        nc, in_maps, core_ids=list(range(8)), trace=trace, **kw
    )
    bo = np.asarray(inputs["bo"], np.float32)
    x = np.asarray(inputs["x"], np.float32)
    Bc = x.shape[0]
    y = np.empty((Bc, L, D), np.float32)
    for b in range(Bc):
        y[b] = res.results[2 * b]["y"] + res.results[2 * b + 1]["y"] + bo
    return y, res


def kernel(**inputs):
    n_heads = int(inputs.get("n_heads", H))
    assert n_heads == H, f"kernel hardcoded for {H} heads, got {n_heads}"
    y, _ = run(inputs, trace=False)
    return y



# revision 11
# speedup vs baseline: 1.1590x; 1.1590x over previous
"""Linformer attention TRN2 kernel (8 NeuronCores), v3.

Sharding: core c handles batch b = c//2 and head-half hh = c%2
(8 of 16 heads = 512 of 1024 feature columns of Wq/Wk/Wv, and the
matching 512 rows of Wo). Each core computes a partial output
y_part = attn_out_half @ Wo[hh*512:(hh+1)*512, :]; the host sums the
two partials per batch and adds bo.

v3 phase 1 uses the Linformer algebra refactor:
    kE = (x Wk + bk)^T E = Wk^T (x^T E) + bk (1^T E)
so the big [L,D]x[D,DH] k/v projections collapse into one shared
z = x^T [E|F] ([D, 2K], accumulated over L in 8 PSUM banks, x as
bf16 stationary / E|F as f32r moving), followed by tiny [D]x[D,K]
projections Wk^T z / zF^T Wv and rank-1 bias matmuls (bk x sum(E),
sum(F) x bv). This cuts phase-1 PE cycles ~2.2x and halves x DMA.

Per-core dataflow (L=4096, D=1024, 8 heads x hd=64, K=256):
  phase 1 (stream L in slices of 512):
    zacc[dc] += xn_chunk^T @ [E|F]_chunk  (8 PSUM banks, 256 matmuls)
    drain z -> SBUF (split ACT/DVE, overlapped with accumulation tail)
    kET[mc] = sum_dc Wk[dc,mc]^T z[dc] + bk[mc] x SE   (PSUM)
    vF[mc]  = sum_dc zF[dc,mc]^T Wv[dc] + SF[mc] x bv  (PSUM)
  phase 2 (per L-tile of 512), software-pipelined (unchanged from v2):
    qT(lt) = Wq_s^T @ xT(lt); scores(h) = qT_h^T @ kET_h; softmax
    (rowmax -> exp accum Z -> normalize); sT = PE-transpose;
    outT_h = vF_h^T @ sT; y(lt) = outT^T @ Wo_s -> DMA.
"""

import sys

sys.path.insert(0, "/opt/trn_rl_repo")

import numpy as np
import ml_dtypes

import concourse.bass as bass
import concourse.mybir as mybir
import concourse.tile as tile
from concourse import bacc
from concourse import bass_utils

B, L, D, H, HD, K = 4, 4096, 1024, 16, 64, 256
DH = 512                      # per-core feature slice (8 heads x 64)
NHL = 8                       # heads per core
SCALE = HD ** -0.5
P = 128
LS1 = 512                     # phase-1 L slice
NS1 = L // LS1                # 8 phase-1 iterations
LT2 = 512                     # phase-2 L tile
NT2 = L // LT2                # 8 phase-2 iterations
F32 = mybir.dt.float32
F32R = mybir.dt.float32r
BF16 = mybir.dt.bfloat16
FP16 = mybir.dt.float16

MMDT = F32R                   # dtype of every matmul-feeding tensor

_CACHE = {}


def build_program():
    nc = bacc.Bacc("TRN2", target_bir_lowering=False, debug=False)

    xt = nc.dram_tensor("xt", [D, L], MMDT, kind="ExternalInput").ap()
    xn = nc.dram_tensor("xn", [L, D], BF16, kind="ExternalInput").ap()
    ef = nc.dram_tensor("ef", [L, 2 * K], BF16, kind="ExternalInput").ap()
    wq = nc.dram_tensor("wq", [D, DH], MMDT, kind="ExternalInput").ap()
    wk = nc.dram_tensor("wk", [D, DH], MMDT, kind="ExternalInput").ap()
    wv = nc.dram_tensor("wv", [D, DH], MMDT, kind="ExternalInput").ap()
    wo = nc.dram_tensor("wo", [DH, D], MMDT, kind="ExternalInput").ap()
    bqs = nc.dram_tensor("bqs", [P, DH // P], F32, kind="ExternalInput").ap()
    # r1 = [bk (512) | SE (256) | SF (256) | bv (512)] rank-1 operands
    r1 = nc.dram_tensor("r1", [1, 1536], MMDT, kind="ExternalInput").ap()
    idbr = nc.dram_tensor("identb", [P, P], BF16, kind="ExternalInput").ap()
    ydr = nc.dram_tensor("y", [L, D], F32, kind="ExternalOutput").ap()

    with tile.TileContext(nc) as tc:
        with (
            tc.tile_pool(name="const", bufs=1) as constp,
            tc.tile_pool(name="persist", bufs=1) as persist,
            tc.tile_pool(name="w2", bufs=1) as w2p,
        ):
            identb_t = constp.tile([P, P], BF16, name="identb_t")
            nc.sync.dma_start(identb_t[:], idbr)
            identb = identb_t[:]
            bqs_sb = constp.tile([P, DH // P], F32)
            nc.sync.dma_start(bqs_sb[:], bqs)
            r1_sb = constp.tile([1, 1536], MMDT)
            nc.sync.dma_start(r1_sb[:], r1)

            # persistent SBUF tensors
            kET_sb = persist.tile([P, 4, K], MMDT)      # [pair-row, pair, k]
            vF_sb = persist.tile([P, 2, DH], BF16)      # [k-in-chunk, kc, h*64+d]

            # phase-2 weights (DMAs emitted later, during phase-1 slices)
            wq_sb = w2p.tile([P, D // P, DH], MMDT, tag="wq")
            wo_sb = w2p.tile([P, DH // P, D], MMDT, tag="wo")

            # ------- phase 1: z = xn^T [E|F]  ->  kET, vF -------
            with tc.tile_pool(name="w1", bufs=1) as w1:
                wk_sb = w1.tile([P, D // P, DH], MMDT, tag="wk")
                wv_sb = w1.tile([P, D // P, DH], MMDT, tag="wv")
                z_sb = w1.tile([P, D // P, 2 * K], MMDT, tag="z")

                with (
                    tc.tile_pool(name="xnp", bufs=2) as xnp,
                    tc.tile_pool(name="efp", bufs=2) as efp,
                    tc.tile_pool(name="ps_z", bufs=1, space="PSUM") as ps_z,
                ):
                    # slice 0 at chunk granularity so the PE can start early
                    xn0 = xnp.tile([P, LS1 // P, D], BF16, tag="xn")
                    ef0 = efp.tile([P, LS1 // P, 2 * K], BF16, tag="ef")
                    for a in range(LS1 // P):
                        nc.sync.dma_start(xn0[:, a, :], xn[a * P : (a + 1) * P, :])
                        nc.sync.dma_start(ef0[:, a, :], ef[a * P : (a + 1) * P, :])
                    # wk/wv ride behind slice 0 (needed only after the z loop)
                    nc.sync.dma_start(wk_sb[:], wk.rearrange("(c p) n -> p c n", p=P))
                    nc.sync.dma_start(wv_sb[:], wv.rearrange("(c p) n -> p c n", p=P))

                    zacc = [
                        ps_z.tile([P, 2 * K], F32, tag=f"z{i}", name=f"zacc{i}")
                        for i in range(D // P)
                    ]

                    for ls in range(NS1):
                        l0 = ls * LS1
                        if ls == 0:
                            xn_sl, ef_sl = xn0, ef0
                        else:
                            xn_sl = xnp.tile([P, LS1 // P, D], BF16, tag="xn")
                            nc.sync.dma_start(
                                xn_sl[:],
                                xn[l0 : l0 + LS1, :].rearrange(
                                    "(a p) d -> p a d", p=P
                                ),
                            )
                            ef_sl = efp.tile([P, LS1 // P, 2 * K], BF16, tag="ef")
                            nc.sync.dma_start(
                                ef_sl[:],
                                ef[l0 : l0 + LS1, :].rearrange(
                                    "(a p) k -> p a k", p=P
                                ),
                            )
                            if ls == 3:
                                nc.sync.dma_start(
                                    wq_sb[:], wq.rearrange("(c p) n -> p c n", p=P)
                                )
                            elif ls == 6:
                                nc.sync.dma_start(
                                    wo_sb[:], wo.rearrange("(c p) n -> p c n", p=P)
                                )
                        first = ls == 0
                        last = ls == NS1 - 1
                        for a in range(LS1 // P):
                            for dc in range(D // P):
                                nc.tensor.matmul(
                                    zacc[dc][:],
                                    xn_sl[:, a, dc * P : (dc + 1) * P],
                                    ef_sl[:, a, :],
                                    start=(first and a == 0),
                                    stop=(last and a == LS1 // P - 1),
                                )
                    # drain z to SBUF, alternating engines; dc0's group stops
                    # 7 matmuls before dc7's so drains overlap the PE tail
                    for dc in range(D // P):
                        if dc % 2 == 0:
                            nc.scalar.copy(z_sb[:, dc, :], zacc[dc][:])
                        else:
                            nc.vector.tensor_copy(z_sb[:, dc, :], zacc[dc][:])

                with tc.tile_pool(name="ps_kv", bufs=1, space="PSUM") as ps_kv:
                    # kET[mc] = sum_dc Wk[dc, mc]^T z[dc] + bk[mc] x SE
                    kps = ps_kv.tile([P, 4, K], F32, tag="kps", name="kps")
                    for mc in range(4):
                        for dc in range(D // P):
                            nc.tensor.matmul(
                                kps[:, mc, :],
                                wk_sb[:, dc, mc * P : (mc + 1) * P],
                                z_sb[:, dc, 0:K],
                                start=(dc == 0),
                                stop=False,
                            )
                        nc.tensor.matmul(
                            kps[:, mc, :],
                            r1_sb[0:1, mc * P : (mc + 1) * P],
                            r1_sb[0:1, 512 : 512 + K],
                            start=False,
                            stop=True,
                        )
                    # vF[mc] = sum_dc zF[dc, mc]^T Wv[dc] + SF[mc] x bv
                    vps = ps_kv.tile([P, 2, DH], F32, tag="vps", name="vps")
                    for mc in range(2):
                        for dc in range(D // P):
                            nc.tensor.matmul(
                                vps[:, mc, :],
                                z_sb[:, dc, K + mc * P : K + (mc + 1) * P],
                                wv_sb[:, dc, :],
                                start=(dc == 0),
                                stop=False,
                            )
                        nc.tensor.matmul(
                            vps[:, mc, :],
                            r1_sb[0:1, 768 + mc * P : 768 + (mc + 1) * P],
                            r1_sb[0:1, 1024:1536],
                            start=False,
                            stop=True,
                        )
                    nc.scalar.copy(kET_sb[:, 0:2, :], kps[:, 0:2, :])
                    nc.scalar.copy(kET_sb[:, 2:4, :], kps[:, 2:4, :])
                    nc.vector.tensor_copy(vF_sb[:, 0, :], vps[:, 0, :])
                    nc.vector.tensor_copy(vF_sb[:, 1, :], vps[:, 1, :])

            # ---------------- phase 2 ----------------
            with (
                tc.tile_pool(name="xt2", bufs=2) as xt2p,
                tc.tile_pool(name="qt", bufs=2) as qtp,
                tc.tile_pool(name="s2p", bufs=8) as s2p,
                tc.tile_pool(name="sTp", bufs=8) as sTp,
                tc.tile_pool(name="otp", bufs=3) as otp,
                tc.tile_pool(name="yp", bufs=4) as yp,
                tc.tile_pool(name="stat", bufs=6) as stat,
                tc.tile_pool(name="ps_q", bufs=1, space="PSUM") as ps_q,
                tc.tile_pool(name="ps_sc", bufs=2, space="PSUM") as ps_sc,
                tc.tile_pool(name="ps_pt", bufs=1, space="PSUM") as ps_pt,
                tc.tile_pool(name="ps_out", bufs=2, space="PSUM") as ps_out,
            ):

                def emit_q_rc(xt_tile, qT_tile, rc):
                    pq = ps_q.tile([P, LT2], F32, tag="pq", name="pq")
                    for dc in range(D // P):
                        nc.tensor.matmul(
                            pq[:],
                            wq_sb[:, dc, rc * P : (rc + 1) * P],
                            xt_tile[:, dc, :],
                            start=(dc == 0),
                            stop=(dc == D // P - 1),
                        )
                    # qT = SCALE*(x@Wq) + SCALE*bq  (bqs pre-scaled on host)
                    nc.scalar.activation(
                        qT_tile[:, rc, :],
                        pq[:],
                        mybir.ActivationFunctionType.Identity,
                        bias=bqs_sb[:, rc : rc + 1],
                        scale=SCALE,
                    )

                def emit_scores(h, qT_t):
                    po = (h % 2) * HD
                    pair = h // 2
                    ps2 = ps_sc.tile([P, 4, K], F32, tag="sc", name="ps2")
                    for j in range(4):
                        nc.tensor.matmul(
                            ps2[:, j, :],
                            qT_t[po : po + HD, pair,
                                 j * P : (j + 1) * P],
                            kET_sb[po : po + HD, pair, :],
                            start=True,
                            stop=True,
                        )
                    negm = stat.tile([P, 4], F32, tag="negm")
                    nc.vector.reduce_max(
                        negm[:], ps2[:], axis=mybir.AxisListType.X, negate=True
                    )
                    s2 = s2p.tile([P, 4, K], BF16, tag="s2")
                    z = stat.tile([P, 4], F32, tag="z")
                    for j in range(4):
                        nc.scalar.activation(
                            s2[:, j, :],
                            ps2[:, j, :],
                            mybir.ActivationFunctionType.Exp,
                            bias=negm[:, j : j + 1],
                            scale=1.0,
                            accum_out=z[:, j : j + 1],
                        )
                    return s2, z

                def emit_zrm(s2, z):
                    rz = stat.tile([P, 4], F32, tag="rz")
                    nc.vector.reciprocal(rz[:], z[:])
                    for j in range(4):
                        nc.vector.tensor_scalar_mul(
                            s2[:, j, :], s2[:, j, :], rz[:, j : j + 1]
                        )

                def emit_transp(h, s2):
                    pt = ps_pt.tile([P, 2, LT2], BF16, tag="pt", name="pt")
                    for j in range(4):
                        for kc in range(2):
                            nc.tensor.transpose(
                                pt[:, kc, j * P : (j + 1) * P],
                                s2[:, j, kc * P : (kc + 1) * P],
                                identb,
                            )
                    sT = sTp.tile([P, 2, LT2], BF16, tag="sT")
                    nc.vector.tensor_copy(sT[:], pt[:])
                    return sT

                def emit_pv(h, sT, outT_t):
                    po = (h % 2) * HD
                    pair = h // 2
                    pp = ps_out.tile([HD, LT2], F32, tag="out", name="pp")
                    for kc in range(2):
                        nc.tensor.matmul(
                            pp[:],
                            vF_sb[:, kc, h * HD : (h + 1) * HD],
                            sT[:, kc, :],
                            start=(kc == 0),
                            stop=(kc == 1),
                        )
                    if h % 2 == 0:
                        nc.vector.tensor_copy(outT_t[po : po + HD, pair, :], pp[:])
                    else:
                        nc.scalar.copy(outT_t[po : po + HD, pair, :], pp[:])

                def emit_y(j, outT_t, l0):
                    y_sb = yp.tile([P, D], F32, tag="y")
                    for nh in range(2):
                        py = ps_out.tile([P, DH], F32, tag="out", name="py")
                        for c in range(4):
                            nc.tensor.matmul(
                                py[:],
                                outT_t[:, c, j * P : (j + 1) * P],
                                wo_sb[:, c, nh * DH : (nh + 1) * DH],
                                start=(c == 0),
                                stop=(c == 3),
                            )
                        if nh == 0:
                            nc.vector.tensor_copy(
                                y_sb[:, nh * DH : (nh + 1) * DH], py[:]
                            )
                        else:
                            nc.scalar.copy(y_sb[:, nh * DH : (nh + 1) * DH], py[:])
                    nc.sync.dma_start(ydr[l0 + j * P : l0 + (j + 1) * P, :], y_sb[:])

                # prologue: q(0)
                xt2_next = xt2p.tile([P, D // P, LT2], MMDT, tag="xt2")
                nc.sync.dma_start(
                    xt2_next[:], xt[:, 0:LT2].rearrange("(c p) l -> p c l", p=P)
                )
                qT_next = qtp.tile([P, 4, LT2], MMDT, tag="qt")
                for rc in range(4):
                    emit_q_rc(xt2_next, qT_next, rc)
                qT_cur = qT_next

                outT_prev = None
                l0_prev = 0
                for lt in range(NT2):
                    l0 = lt * LT2
                    if lt + 1 < NT2:
                        xt2_next = xt2p.tile([P, D // P, LT2], MMDT, tag="xt2")
                        nc.sync.dma_start(
                            xt2_next[:],
                            xt[:, l0 + LT2 : l0 + 2 * LT2].rearrange(
                                "(c p) l -> p c l", p=P
                            ),
                        )
                    have_q = lt + 1 < NT2
                    have_y = outT_prev is not None
                    if have_q:
                        qT_next = qtp.tile([P, 4, LT2], MMDT, tag="qt")

                    # spread schedule: scores trickle out at the rate ACT
                    # can exp-drain them (2 PSUM score buffers); transposes,
                    # q(lt+1) and y(lt-1) fill the PE gaps.
                    s2s = []
                    sTs = []
                    s2s.append(emit_scores(0, qT_cur))
                    s2s.append(emit_scores(1, qT_cur))
                    if have_y:
                        emit_y(0, outT_prev, l0_prev)
                        emit_y(1, outT_prev, l0_prev)
                    s2s.append(emit_scores(2, qT_cur))
                    emit_zrm(*s2s[0])
                    sTs.append(emit_transp(0, s2s[0][0]))
                    s2s.append(emit_scores(3, qT_cur))
                    emit_zrm(*s2s[1])
                    sTs.append(emit_transp(1, s2s[1][0]))
                    outT_t = otp.tile([P, 4, LT2], MMDT, tag="outT")
                    s2s.append(emit_scores(4, qT_cur))
                    emit_zrm(*s2s[2])
                    sTs.append(emit_transp(2, s2s[2][0]))
                    if have_q:
                        emit_q_rc(xt2_next, qT_next, 0)
                    emit_pv(0, sTs[0], outT_t)
                    s2s.append(emit_scores(5, qT_cur))
                    emit_zrm(*s2s[3])
                    sTs.append(emit_transp(3, s2s[3][0]))
                    if have_q:
                        emit_q_rc(xt2_next, qT_next, 1)
                    emit_pv(1, sTs[1], outT_t)
                    s2s.append(emit_scores(6, qT_cur))
                    emit_zrm(*s2s[4])
                    sTs.append(emit_transp(4, s2s[4][0]))
                    if have_q:
                        emit_q_rc(xt2_next, qT_next, 2)
                    emit_pv(2, sTs[2], outT_t)
                    s2s.append(emit_scores(7, qT_cur))
                    emit_zrm(*s2s[5])
                    sTs.append(emit_transp(5, s2s[5][0]))
                    if have_q:
                        emit_q_rc(xt2_next, qT_next, 3)
                    emit_pv(3, sTs[3], outT_t)
                    emit_zrm(*s2s[6])
                    sTs.append(emit_transp(6, s2s[6][0]))
                    emit_pv(4, sTs[4], outT_t)
                    emit_zrm(*s2s[7])
                    sTs.append(emit_transp(7, s2s[7][0]))
                    emit_pv(5, sTs[5], outT_t)
                    emit_pv(6, sTs[6], outT_t)
                    emit_pv(7, sTs[7], outT_t)
                    # y(lt-1) j2-3 (executes while PV of this tile runs)
                    if have_y:
                        emit_y(2, outT_prev, l0_prev)
                        emit_y(3, outT_prev, l0_prev)
                    outT_prev = outT_t
                    l0_prev = l0
                    if have_q:
                        qT_cur = qT_next
                # epilogue: y(7)
                for j in range(4):
                    emit_y(j, outT_prev, l0_prev)
    nc.compile()
    return nc


def _get_program():
    if "nc" not in _CACHE:
        _CACHE["nc"] = build_program()
    return _CACHE["nc"]


def _shard_inputs(inputs):
    x = np.asarray(inputs["x"], np.float32)
    Wq = np.asarray(inputs["Wq"], np.float32)
    bq = np.asarray(inputs["bq"], np.float32)
    Wk = np.asarray(inputs["Wk"], np.float32)
    bk = np.asarray(inputs["bk"], np.float32)
    Wv = np.asarray(inputs["Wv"], np.float32)
    bv = np.asarray(inputs["bv"], np.float32)
    E = np.ascontiguousarray(np.asarray(inputs["E"], np.float32))
    F = np.ascontiguousarray(np.asarray(inputs["F"], np.float32))
    Wo = np.asarray(inputs["Wo"], np.float32)
    ef = np.ascontiguousarray(np.hstack([E, F]).astype(ml_dtypes.bfloat16))
    SE, SF = E.sum(0), F.sum(0)
    identb = np.eye(P, dtype=ml_dtypes.bfloat16)
    xns = [np.ascontiguousarray(x[b]).astype(ml_dtypes.bfloat16) for b in range(4)]
    xts = [np.ascontiguousarray(x[b].T) for b in range(4)]
    in_maps = []
    for c in range(8):
        b, hh = c // 2, c % 2
        sl = slice(hh * DH, (hh + 1) * DH)
        r1 = np.concatenate([bk[sl], SE, SF, bv[sl]]).reshape(1, 1536)
        in_maps.append(
            {
                "xt": xts[b],
                "xn": xns[b],
                "ef": ef,
                "wq": np.ascontiguousarray(Wq[:, sl]),
                "wk": np.ascontiguousarray(Wk[:, sl]),
                "wv": np.ascontiguousarray(Wv[:, sl]),
                "wo": np.ascontiguousarray(Wo[sl, :]),
                "bqs": np.ascontiguousarray(
                    (bq[sl] * SCALE).reshape(4, P).T.astype(np.float32)
                ),
                "r1": np.ascontiguousarray(r1.astype(np.float32)),
                "identb": identb,
            }
        )
    return in_maps


def _ensure_profile_hook():
    """The container's `antenv` stub lacks `axon_hooks`; synthesize it so
    run_bass_kernel_spmd(trace=True) can reach the NTFF capture ABI in
    libaxon_pjrt.so (see trn_agent_boot.trn_boot)."""
    import types
    import antenv

    if hasattr(antenv, "axon_hooks"):
        return
    mod = types.ModuleType("antenv.axon_hooks")
    _state = {"hook": None}
    mod.set_axon_ntff_profile_hook = lambda h: _state.__setitem__("hook", h)
    mod.get_axon_ntff_profile_hook = lambda: _state["hook"]
    sys.modules["antenv.axon_hooks"] = mod
    antenv.axon_hooks = mod
    try:
        from trn_agent_boot.trn_boot import _ntff_profile_via_ctypes

        mod.set_axon_ntff_profile_hook(
            _ntff_profile_via_ctypes("/opt/axon/libaxon_pjrt.so")
        )
    except Exception as e:
        print(f"profile hook setup failed: {e}", file=sys.stderr)


def run(inputs, trace=False, **kw):
    if trace:
        _ensure_profile_hook()
    nc = _get_program()
    in_maps = _shard_inputs(inputs)
    res = bass_utils.run_bass_kernel_spmd(
        nc, in_maps, core_ids=list(range(8)), trace=trace, **kw
    )
    bo = np.asarray(inputs["bo"], np.float32)
    x = np.asarray(inputs["x"], np.float32)
    Bc = x.shape[0]
    y = np.empty((Bc, L, D), np.float32)
    for b in range(Bc):
        y[b] = res.results[2 * b]["y"] + res.results[2 * b + 1]["y"] + bo
    return y, res


def kernel(**inputs):
    n_heads = int(inputs.get("n_heads", H))
    assert n_heads == H, f"kernel hardcoded for {H} heads, got {n_heads}"
    y, _ = run(inputs, trace=False)
    return y



# revision 19
# speedup vs baseline: 1.1809x; 1.0189x over previous
"""Linformer attention TRN2 kernel (8 NeuronCores), v3.

Sharding: core c handles batch b = c//2 and head-half hh = c%2
(8 of 16 heads = 512 of 1024 feature columns of Wq/Wk/Wv, and the
matching 512 rows of Wo). Each core computes a partial output
y_part = attn_out_half @ Wo[hh*512:(hh+1)*512, :]; the host sums the
two partials per batch and adds bo.

v3 phase 1 uses the Linformer algebra refactor:
    kE = (x Wk + bk)^T E = Wk^T (x^T E) + bk (1^T E)
so the big [L,D]x[D,DH] k/v projections collapse into one shared
z = x^T [E|F] ([D, 2K], accumulated over L in 8 PSUM banks, x as
bf16 stationary / E|F as f32r moving), followed by tiny [D]x[D,K]
projections Wk^T z / zF^T Wv and rank-1 bias matmuls (bk x sum(E),
sum(F) x bv). This cuts phase-1 PE cycles ~2.2x and halves x DMA.

Per-core dataflow (L=4096, D=1024, 8 heads x hd=64, K=256):
  phase 1 (stream L in slices of 512):
    zacc[dc] += xn_chunk^T @ [E|F]_chunk  (8 PSUM banks, 256 matmuls)
    drain z -> SBUF (split ACT/DVE, overlapped with accumulation tail)
    kET[mc] = sum_dc Wk[dc,mc]^T z[dc] + bk[mc] x SE   (PSUM)
    vF[mc]  = sum_dc zF[dc,mc]^T Wv[dc] + SF[mc] x bv  (PSUM)
  phase 2 (per L-tile of 512), software-pipelined (unchanged from v2):
    qT(lt) = Wq_s^T @ xT(lt); scores(h) = qT_h^T @ kET_h; softmax
    (rowmax -> exp accum Z -> normalize); sT = PE-transpose;
    outT_h = vF_h^T @ sT; y(lt) = outT^T @ Wo_s -> DMA.
"""

import sys

sys.path.insert(0, "/opt/trn_rl_repo")

import numpy as np
import ml_dtypes

import concourse.bass as bass
import concourse.mybir as mybir
import concourse.tile as tile
from concourse import bacc
from concourse import bass_utils

B, L, D, H, HD, K = 4, 4096, 1024, 16, 64, 256
DH = 512                      # per-core feature slice (8 heads x 64)
NHL = 8                       # heads per core
SCALE = HD ** -0.5
P = 128
LS1 = 512                     # phase-1 L slice
NS1 = L // LS1                # 8 phase-1 iterations
LT2 = 512                     # phase-2 L tile
NT2 = L // LT2                # 8 phase-2 iterations
F32 = mybir.dt.float32
F32R = mybir.dt.float32r
BF16 = mybir.dt.bfloat16
FP16 = mybir.dt.float16

MMDT = F32R                   # dtype of every matmul-feeding tensor

_CACHE = {}


def build_program():
    nc = bacc.Bacc("TRN2", target_bir_lowering=False, debug=False)

    # pre-swizzled on host so every DMA is a long contiguous run per
    # partition: xt[p, t, c, :] = x^T[c*128+p, t*512:(t+1)*512] etc.
    xt = nc.dram_tensor("xt", [P, NT2, D // P, LT2], MMDT, kind="ExternalInput").ap()
    xn = nc.dram_tensor("xn", [P, L // P, D], BF16, kind="ExternalInput").ap()
    ef = nc.dram_tensor("ef", [P, L // P, 2 * K], BF16, kind="ExternalInput").ap()
    wq = nc.dram_tensor("wq", [D, DH], MMDT, kind="ExternalInput").ap()
    wk = nc.dram_tensor("wk", [D, DH], MMDT, kind="ExternalInput").ap()
    wv = nc.dram_tensor("wv", [D, DH], MMDT, kind="ExternalInput").ap()
    wo = nc.dram_tensor("wo", [DH, D], MMDT, kind="ExternalInput").ap()
    bqs = nc.dram_tensor("bqs", [P, DH // P], F32, kind="ExternalInput").ap()
    # r1 = [bk (512) | SE (256) | SF (256) | bv (512)] rank-1 operands
    r1 = nc.dram_tensor("r1", [1, 1536], MMDT, kind="ExternalInput").ap()
    idbr = nc.dram_tensor("identb", [P, P], BF16, kind="ExternalInput").ap()
    ydr = nc.dram_tensor("y", [L, D], F32, kind="ExternalOutput").ap()

    with tile.TileContext(nc) as tc:
        with (
            tc.tile_pool(name="const", bufs=1) as constp,
            tc.tile_pool(name="persist", bufs=1) as persist,
            tc.tile_pool(name="w2", bufs=1) as w2p,
        ):
            # persistent SBUF tensors
            kET_sb = persist.tile([P, 4, K], MMDT)      # [pair-row, pair, k]
            vF_sb = persist.tile([P, 2, DH], BF16)      # [k-in-chunk, kc, h*64+d]

            # phase-2 weights (DMAs emitted later, during phase-1 slices)
            wq_sb = w2p.tile([P, D // P, DH], MMDT, tag="wq")
            wo_sb = w2p.tile([P, DH // P, D], MMDT, tag="wo")

            # phase-2 xT tile pool lives at outer scope so tile 0 can be
            # prefetched during phase 1
            xt2p_cm = tc.tile_pool(name="xt2", bufs=2)
            xt2p = xt2p_cm.__enter__()

            # ------- phase 1: z = xn^T [E|F]  ->  kET, vF -------
            with tc.tile_pool(name="w1", bufs=1) as w1:
                wk_sb = w1.tile([P, D // P, DH], MMDT, tag="wk")
                wv_sb = w1.tile([P, D // P, DH], MMDT, tag="wv")
                z_sb = w1.tile([P, D // P, 2 * K], MMDT, tag="z")

                with (
                    tc.tile_pool(name="xnp", bufs=2) as xnp,
                    tc.tile_pool(name="efp", bufs=2) as efp,
                    tc.tile_pool(name="ps_z", bufs=1, space="PSUM") as ps_z,
                ):
                    # slice 0 at chunk granularity so the PE can start early
                    xn0 = xnp.tile([P, LS1 // P, D], BF16, tag="xn")
                    ef0 = efp.tile([P, LS1 // P, 2 * K], BF16, tag="ef")
                    for a in range(LS1 // P):
                        nc.sync.dma_start(xn0[:, a, :], xn[:, a, :])
                        nc.sync.dma_start(ef0[:, a, :], ef[:, a, :])
                    # small constants ride behind the first slice
                    identb_t = constp.tile([P, P], BF16, name="identb_t")
                    nc.sync.dma_start(identb_t[:], idbr)
                    identb = identb_t[:]
                    bqs_sb = constp.tile([P, DH // P], F32)
                    nc.sync.dma_start(bqs_sb[:], bqs)
                    r1_sb = constp.tile([1, 1536], MMDT)
                    nc.sync.dma_start(r1_sb[:], r1)

                    zacc = [
                        ps_z.tile([P, 2 * K], F32, tag=f"z{i}", name=f"zacc{i}")
                        for i in range(D // P)
                    ]

                    for ls in range(NS1):
                        if ls > 0:
                            a0 = ls * (LS1 // P)
                            xn_sl = xnp.tile([P, LS1 // P, D], BF16, tag="xn")
                            nc.sync.dma_start(
                                xn_sl[:], xn[:, a0 : a0 + LS1 // P, :]
                            )
                            ef_sl = efp.tile([P, LS1 // P, 2 * K], BF16, tag="ef")
                            nc.sync.dma_start(
                                ef_sl[:], ef[:, a0 : a0 + LS1 // P, :]
                            )
                            # weight loads interleave with the stream; all are
                            # needed only at/after the z-loop tail
                            if ls == 1:
                                nc.sync.dma_start(
                                    wk_sb[:], wk.rearrange("(c p) n -> p c n", p=P)
                                )
                            elif ls == 2:
                                nc.sync.dma_start(
                                    wv_sb[:], wv.rearrange("(c p) n -> p c n", p=P)
                                )
                            elif ls == 3:
                                nc.sync.dma_start(
                                    wq_sb[:], wq.rearrange("(c p) n -> p c n", p=P)
                                )
                            elif ls == 5:
                                # prefetch phase-2 tile 0 xT
                                xt2_pre = xt2p.tile(
                                    [P, D // P, LT2], MMDT, tag="xt2"
                                )
                                nc.sync.dma_start(xt2_pre[:], xt[:, 0, :, :])
                            elif ls == 6:
                                nc.sync.dma_start(
                                    wo_sb[:], wo.rearrange("(c p) n -> p c n", p=P)
                                )
                        else:
                            xn_sl, ef_sl = xn0, ef0
                        first = ls == 0
                        last = ls == NS1 - 1
                        for a in range(LS1 // P):
                            for dc in range(D // P):
                                nc.tensor.matmul(
                                    zacc[dc][:],
                                    xn_sl[:, a, dc * P : (dc + 1) * P],
                                    ef_sl[:, a, :],
                                    start=(first and a == 0),
                                    stop=(last and a == LS1 // P - 1),
                                )
                    # drain z to SBUF, alternating engines; dc0's group stops
                    # 7 matmuls before dc7's so drains overlap the PE tail
                    for dc in range(D // P):
                        if dc % 2 == 0:
                            nc.scalar.copy(z_sb[:, dc, :], zacc[dc][:])
                        else:
                            nc.vector.tensor_copy(z_sb[:, dc, :], zacc[dc][:])

                with tc.tile_pool(name="ps_kv", bufs=1, space="PSUM") as ps_kv:
                    # kET[mc] = sum_dc Wk[dc, mc]^T z[dc] + bk[mc] x SE
                    kps = ps_kv.tile([P, 4, K], F32, tag="kps", name="kps")
                    for mc in range(4):
                        for dc in range(D // P):
                            nc.tensor.matmul(
                                kps[:, mc, :],
                                wk_sb[:, dc, mc * P : (mc + 1) * P],
                                z_sb[:, dc, 0:K],
                                start=(dc == 0),
                                stop=False,
                            )
                        nc.tensor.matmul(
                            kps[:, mc, :],
                            r1_sb[0:1, mc * P : (mc + 1) * P],
                            r1_sb[0:1, 512 : 512 + K],
                            start=False,
                            stop=True,
                        )
                    # vF[mc] = sum_dc zF[dc, mc]^T Wv[dc] + SF[mc] x bv
                    vps = ps_kv.tile([P, 2, DH], F32, tag="vps", name="vps")
                    for mc in range(2):
                        for dc in range(D // P):
                            nc.tensor.matmul(
                                vps[:, mc, :],
                                z_sb[:, dc, K + mc * P : K + (mc + 1) * P],
                                wv_sb[:, dc, :],
                                start=(dc == 0),
                                stop=False,
                            )
                        nc.tensor.matmul(
                            vps[:, mc, :],
                            r1_sb[0:1, 768 + mc * P : 768 + (mc + 1) * P],
                            r1_sb[0:1, 1024:1536],
                            start=False,
                            stop=True,
                        )
                    nc.scalar.copy(kET_sb[:, 0:2, :], kps[:, 0:2, :])
                    nc.scalar.copy(kET_sb[:, 2:4, :], kps[:, 2:4, :])
                    nc.vector.tensor_copy(vF_sb[:, 0, :], vps[:, 0, :])
                    nc.vector.tensor_copy(vF_sb[:, 1, :], vps[:, 1, :])

            # ---------------- phase 2 ----------------
            with (
                tc.tile_pool(name="qt", bufs=2) as qtp,
                tc.tile_pool(name="s2p", bufs=8) as s2p,
                tc.tile_pool(name="sTp", bufs=8) as sTp,
                tc.tile_pool(name="otp", bufs=3) as otp,
                tc.tile_pool(name="yp", bufs=4) as yp,
                tc.tile_pool(name="stat", bufs=6) as stat,
                tc.tile_pool(name="ps_q", bufs=1, space="PSUM") as ps_q,
                tc.tile_pool(name="ps_sc", bufs=2, space="PSUM") as ps_sc,
                tc.tile_pool(name="ps_pt", bufs=1, space="PSUM") as ps_pt,
                tc.tile_pool(name="ps_out", bufs=2, space="PSUM") as ps_out,
            ):

                def emit_q_rc(xt_tile, qT_tile, rc):
                    pq = ps_q.tile([P, LT2], F32, tag="pq", name="pq")
                    for dc in range(D // P):
                        nc.tensor.matmul(
                            pq[:],
                            wq_sb[:, dc, rc * P : (rc + 1) * P],
                            xt_tile[:, dc, :],
                            start=(dc == 0),
                            stop=(dc == D // P - 1),
                        )
                    # qT = SCALE*(x@Wq) + SCALE*bq  (bqs pre-scaled on host)
                    nc.scalar.activation(
                        qT_tile[:, rc, :],
                        pq[:],
                        mybir.ActivationFunctionType.Identity,
                        bias=bqs_sb[:, rc : rc + 1],
                        scale=SCALE,
                    )

                def emit_scores(h, qT_t):
                    po = (h % 2) * HD
                    pair = h // 2
                    ps2 = ps_sc.tile([P, 4, K], F32, tag="sc", name="ps2")
                    for j in range(4):
                        nc.tensor.matmul(
                            ps2[:, j, :],
                            qT_t[po : po + HD, pair,
                                 j * P : (j + 1) * P],
                            kET_sb[po : po + HD, pair, :],
                            start=True,
                            stop=True,
                        )
                    negm = stat.tile([P, 4], F32, tag="negm")
                    nc.vector.reduce_max(
                        negm[:], ps2[:], axis=mybir.AxisListType.X, negate=True
                    )
                    s2 = s2p.tile([P, 4, K], BF16, tag="s2")
                    z = stat.tile([P, 4], F32, tag="z")
                    for j in range(4):
                        nc.scalar.activation(
                            s2[:, j, :],
                            ps2[:, j, :],
                            mybir.ActivationFunctionType.Exp,
                            bias=negm[:, j : j + 1],
                            scale=1.0,
                            accum_out=z[:, j : j + 1],
                        )
                    return s2, z

                def emit_zrm(s2, z):
                    rz = stat.tile([P, 4], F32, tag="rz")
                    nc.vector.reciprocal(rz[:], z[:])
                    for j in range(4):
                        nc.vector.tensor_scalar_mul(
                            s2[:, j, :], s2[:, j, :], rz[:, j : j + 1]
                        )

                def emit_transp(h, s2):
                    pt = ps_pt.tile([P, 2, LT2], BF16, tag="pt", name="pt")
                    for j in range(4):
                        for kc in range(2):
                            nc.tensor.transpose(
                                pt[:, kc, j * P : (j + 1) * P],
                                s2[:, j, kc * P : (kc + 1) * P],
                                identb,
                            )
                    sT = sTp.tile([P, 2, LT2], BF16, tag="sT")
                    nc.vector.tensor_copy(sT[:], pt[:])
                    return sT

                def emit_pv(h, sT, outT_t):
                    po = (h % 2) * HD
                    pair = h // 2
                    pp = ps_out.tile([HD, LT2], F32, tag="out", name="pp")
                    for kc in range(2):
                        nc.tensor.matmul(
                            pp[:],
                            vF_sb[:, kc, h * HD : (h + 1) * HD],
                            sT[:, kc, :],
                            start=(kc == 0),
                            stop=(kc == 1),
                        )
                    if h % 2 == 0:
                        nc.vector.tensor_copy(outT_t[po : po + HD, pair, :], pp[:])
                    else:
                        nc.scalar.copy(outT_t[po : po + HD, pair, :], pp[:])

                def emit_y(j, outT_t, l0):
                    y_sb = yp.tile([P, D], F32, tag="y")
                    for nh in range(2):
                        py = ps_out.tile([P, DH], F32, tag="out", name="py")
                        for c in range(4):
                            nc.tensor.matmul(
                                py[:],
                                outT_t[:, c, j * P : (j + 1) * P],
                                wo_sb[:, c, nh * DH : (nh + 1) * DH],
                                start=(c == 0),
                                stop=(c == 3),
                            )
                        if nh == 0:
                            nc.vector.tensor_copy(
                                y_sb[:, nh * DH : (nh + 1) * DH], py[:]
                            )
                        else:
                            nc.scalar.copy(y_sb[:, nh * DH : (nh + 1) * DH], py[:])
                        # ship each half as soon as its drain lands
                        nc.sync.dma_start(
                            ydr[l0 + j * P : l0 + (j + 1) * P,
                                nh * DH : (nh + 1) * DH],
                            y_sb[:, nh * DH : (nh + 1) * DH],
                        )

                # prologue: q(0) — xT tile 0 was prefetched in phase 1
                xt2_next = xt2_pre
                qT_next = qtp.tile([P, 4, LT2], MMDT, tag="qt")
                for rc in range(4):
                    emit_q_rc(xt2_next, qT_next, rc)
                qT_cur = qT_next

                outT_prev = None
                l0_prev = 0
                for lt in range(NT2):
                    l0 = lt * LT2
                    if lt + 1 < NT2:
                        xt2_next = xt2p.tile([P, D // P, LT2], MMDT, tag="xt2")
                        nc.sync.dma_start(xt2_next[:], xt[:, lt + 1, :, :])
                    have_q = lt + 1 < NT2
                    have_y = outT_prev is not None
                    if have_q:
                        qT_next = qtp.tile([P, 4, LT2], MMDT, tag="qt")

                    # spread schedule: scores trickle out at the rate ACT
                    # can exp-drain them (2 PSUM score buffers); transposes,
                    # q(lt+1) and y(lt-1) fill the PE gaps.
                    s2s = []
                    sTs = []
                    s2s.append(emit_scores(0, qT_cur))
                    s2s.append(emit_scores(1, qT_cur))
                    if have_y:
                        emit_y(0, outT_prev, l0_prev)
                        emit_y(1, outT_prev, l0_prev)
                    s2s.append(emit_scores(2, qT_cur))
                    emit_zrm(*s2s[0])
                    sTs.append(emit_transp(0, s2s[0][0]))
                    s2s.append(emit_scores(3, qT_cur))
                    emit_zrm(*s2s[1])
                    sTs.append(emit_transp(1, s2s[1][0]))
                    outT_t = otp.tile([P, 4, LT2], MMDT, tag="outT")
                    s2s.append(emit_scores(4, qT_cur))
                    emit_zrm(*s2s[2])
                    sTs.append(emit_transp(2, s2s[2][0]))
                    if have_q:
                        emit_q_rc(xt2_next, qT_next, 0)
                    emit_pv(0, sTs[0], outT_t)
                    s2s.append(emit_scores(5, qT_cur))
                    emit_zrm(*s2s[3])
                    sTs.append(emit_transp(3, s2s[3][0]))
                    if have_q:
                        emit_q_rc(xt2_next, qT_next, 1)
                    emit_pv(1, sTs[1], outT_t)
                    s2s.append(emit_scores(6, qT_cur))
                    emit_zrm(*s2s[4])
                    sTs.append(emit_transp(4, s2s[4][0]))
                    if have_q:
                        emit_q_rc(xt2_next, qT_next, 2)
                    emit_pv(2, sTs[2], outT_t)
                    s2s.append(emit_scores(7, qT_cur))
                    emit_zrm(*s2s[5])
                    sTs.append(emit_transp(5, s2s[5][0]))
                    if have_q:
                        emit_q_rc(xt2_next, qT_next, 3)
                    emit_pv(3, sTs[3], outT_t)
                    emit_zrm(*s2s[6])
                    sTs.append(emit_transp(6, s2s[6][0]))
                    emit_pv(4, sTs[4], outT_t)
                    emit_zrm(*s2s[7])
                    sTs.append(emit_transp(7, s2s[7][0]))
                    emit_pv(5, sTs[5], outT_t)
                    emit_pv(6, sTs[6], outT_t)
                    emit_pv(7, sTs[7], outT_t)
                    # y(lt-1) j2-3 (executes while PV of this tile runs)
                    if have_y:
                        emit_y(2, outT_prev, l0_prev)
                        emit_y(3, outT_prev, l0_prev)
                    outT_prev = outT_t
                    l0_prev = l0
                    if have_q:
                        qT_cur = qT_next
                # epilogue: y(7)
                for j in range(4):
                    emit_y(j, outT_prev, l0_prev)
            xt2p_cm.__exit__(None, None, None)
    nc.compile()
    return nc


def _get_program():
    if "nc" not in _CACHE:
        _CACHE["nc"] = build_program()
    return _CACHE["nc"]


def _shard_inputs(inputs):
    x = np.asarray(inputs["x"], np.float32)
    Wq = np.asarray(inputs["Wq"], np.float32)
    bq = np.asarray(inputs["bq"], np.float32)
    Wk = np.asarray(inputs["Wk"], np.float32)
    bk = np.asarray(inputs["bk"], np.float32)
    Wv = np.asarray(inputs["Wv"], np.float32)
    bv = np.asarray(inputs["bv"], np.float32)
    E = np.ascontiguousarray(np.asarray(inputs["E"], np.float32))
    F = np.ascontiguousarray(np.asarray(inputs["F"], np.float32))
    Wo = np.asarray(inputs["Wo"], np.float32)
    # swizzled layouts: partition-major so DMAs read long contiguous runs
    # ef[p, i, :] = [E|F][i*128 + p, :]
    ef = np.ascontiguousarray(
        np.hstack([E, F]).astype(ml_dtypes.bfloat16)
        .reshape(L // P, P, 2 * K).transpose(1, 0, 2)
    )
    SE, SF = E.sum(0), F.sum(0)
    identb = np.eye(P, dtype=ml_dtypes.bfloat16)
    # xn[p, i, :] = x[b][i*128 + p, :]
    xns = [
        np.ascontiguousarray(
            x[b].astype(ml_dtypes.bfloat16)
            .reshape(L // P, P, D).transpose(1, 0, 2)
        )
        for b in range(4)
    ]
    # xt[p, t, c, :] = x[b].T[c*128 + p, t*512:(t+1)*512]
    xts = [
        np.ascontiguousarray(
            x[b].T.reshape(D // P, P, L // 512, 512).transpose(1, 2, 0, 3)
        )
        for b in range(4)
    ]
    in_maps = []
    for c in range(8):
        b, hh = c // 2, c % 2
        sl = slice(hh * DH, (hh + 1) * DH)
        r1 = np.concatenate([bk[sl], SE, SF, bv[sl]]).reshape(1, 1536)
        in_maps.append(
            {
                "xt": xts[b],
                "xn": xns[b],
                "ef": ef,
                "wq": np.ascontiguousarray(Wq[:, sl]),
                "wk": np.ascontiguousarray(Wk[:, sl]),
                "wv": np.ascontiguousarray(Wv[:, sl]),
                "wo": np.ascontiguousarray(Wo[sl, :]),
                "bqs": np.ascontiguousarray(
                    (bq[sl] * SCALE).reshape(4, P).T.astype(np.float32)
                ),
                "r1": np.ascontiguousarray(r1.astype(np.float32)),
                "identb": identb,
            }
        )
    return in_maps


def _ensure_profile_hook():
    """The container's `antenv` stub lacks `axon_hooks`; synthesize it so
    run_bass_kernel_spmd(trace=True) can reach the NTFF capture ABI in
    libaxon_pjrt.so (see trn_agent_boot.trn_boot)."""
    import types
    import antenv

    if hasattr(antenv, "axon_hooks"):
        return
    mod = types.ModuleType("antenv.axon_hooks")
    _state = {"hook": None}
    mod.set_axon_ntff_profile_hook = lambda h: _state.__setitem__("hook", h)
    mod.get_axon_ntff_profile_hook = lambda: _state["hook"]
    sys.modules["antenv.axon_hooks"] = mod
    antenv.axon_hooks = mod
    try:
        from trn_agent_boot.trn_boot import _ntff_profile_via_ctypes

        mod.set_axon_ntff_profile_hook(
            _ntff_profile_via_ctypes("/opt/axon/libaxon_pjrt.so")
        )
    except Exception as e:
        print(f"profile hook setup failed: {e}", file=sys.stderr)


def run(inputs, trace=False, **kw):
    if trace:
        _ensure_profile_hook()
    nc = _get_program()
    in_maps = _shard_inputs(inputs)
    res = bass_utils.run_bass_kernel_spmd(
        nc, in_maps, core_ids=list(range(8)), trace=trace, **kw
    )
    bo = np.asarray(inputs["bo"], np.float32)
    x = np.asarray(inputs["x"], np.float32)
    Bc = x.shape[0]
    y = np.empty((Bc, L, D), np.float32)
    for b in range(Bc):
        y[b] = res.results[2 * b]["y"] + res.results[2 * b + 1]["y"] + bo
    return y, res


def kernel(**inputs):
    n_heads = int(inputs.get("n_heads", H))
    assert n_heads == H, f"kernel hardcoded for {H} heads, got {n_heads}"
    y, _ = run(inputs, trace=False)
    return y

